# revision 22
# baseline (speedup 1.0000x reference)
"""BiGRU encoder on 8 Trainium2 NeuronCores.

Strategy: the T=2048 recurrence is split into 32 chunks per direction of 64
steps each, computed in parallel as independent chains with a 32-step warm-up
prefix (the GRU state's dependence on its past decays geometrically; W=32
gives ~5e-3 relative error vs an exact scan). Cores 0-3 run the forward
direction (8 chains x 16 batch = 128 rows each), cores 4-7 the backward
direction on host-reversed data. Per step, each core does:
  gates = [x_t | h_{t-1}] @ [Wih | Whh]^T  as bf16 matmuls (stationary = xT /
  hT chunks of 128 rows, moving = weight tiles [128,512]), accumulated in
  f32 PSUM; sigmoid/tanh on ACT (bf16 gates); GRU update on DVE with the
  state h kept in f32 (bf16 state lets quantization noise compound through
  the residual stream). h is transposed for the next step's matmul with
  PE-transpose; the psum->sbuf hT copies run on DVE (casting to bf16) so the
  ACT queue stays short. Next-step gi matmuls are interleaved into the
  recurrent matmul sequence as fill so the PE never stalls at the head of
  its queue.
The host slices x, builds the per-core layouts, and reassembles the output.
"""
import os
import sys
import numpy as np
import ml_dtypes

try:
    import concourse.bass as bass
except ImportError:
    import sys
    sys.path.insert(0, "/opt/trn_rl_repo")
    import concourse.bass as bass

import concourse.tile as tile
from concourse import bacc, mybir
from concourse.bass_utils import run_bass_kernel_spmd

F32 = mybir.dt.float32
BF16 = mybir.dt.bfloat16
NP_BF16 = ml_dtypes.bfloat16

# geometry (hardcoded for this problem)
B = 16          # batch
T = 2048        # timesteps
F = 512         # hidden/feature size
H = F // 2
KC = 4          # contraction chunks (F / 128)
CHUNK = int(os.environ.get("GRU_CHUNK", "64"))   # stored steps per chain
WARM = int(os.environ.get("GRU_WARM", "30"))     # warm-up steps per chain
S = CHUNK + WARM                                  # total steps per core
NCH = 8         # chains per core
R = NCH * B     # rows per core = 128
N_CORES = 8
N_FWD = 4       # cores 0..3 forward, 4..7 backward
ACT = mybir.ActivationFunctionType
ALU = mybir.AluOpType

_PROG_CACHE = {}


def _build_program(has_bias: bool):
    nc = bacc.Bacc("TRN2", target_bir_lowering=False, debug=False)

    xT_d = nc.dram_tensor("xT", [S, 128, KC, 128], BF16, kind="ExternalInput").ap()
    xr_d = nc.dram_tensor("xr", [S, 128, F], BF16, kind="ExternalInput").ap()
    wih_d = nc.dram_tensor("wih", [128, KC, 3 * F], BF16, kind="ExternalInput").ap()
    whh_d = nc.dram_tensor("whh", [128, KC, 3 * F], BF16, kind="ExternalInput").ap()
    ident_d = nc.dram_tensor("ident", [128, 128], F32, kind="ExternalInput").ap()
    if has_bias:
        bias_i_d = nc.dram_tensor("bias_i", [1, 3 * F], BF16, kind="ExternalInput").ap()
        bias_h_d = nc.dram_tensor("bias_h", [1, 3 * F], BF16, kind="ExternalInput").ap()
        ones_d = nc.dram_tensor("ones", [1, 128], BF16, kind="ExternalInput").ap()
    out_d = nc.dram_tensor("out", [CHUNK, 128, F], F32, kind="ExternalOutput").ap()

    with tile.TileContext(nc) as tc:
        with (
            tc.tile_pool(name="const", bufs=1) as constp,
            tc.tile_pool(name="xs", bufs=1) as xsp,
            tc.tile_pool(name="ew", bufs=1) as ewp,
            tc.tile_pool(name="ps", bufs=1, space="PSUM") as psp,
        ):
            # load wih gate-by-gate so the first gi matmuls can start as
            # soon as the r-slice + xT(0) land rather than after the full 3MB
            wih = constp.tile([128, KC, 3 * F], BF16, name="wih_sb")
            whh = constp.tile([128, KC, 3 * F], BF16, name="whh_sb")
            ident = constp.tile([128, 128], F32, name="ident_sb")
            nc.sync.dma_start(wih[:, :, 0:F], wih_d[:, :, 0:F])
            if has_bias:
                bias_i = constp.tile([1, 3 * F], BF16, name="bias_i_sb")
                nc.sync.dma_start(bias_i[:], bias_i_d[:])
                bias_h = constp.tile([1, 3 * F], BF16, name="bias_h_sb")
                nc.sync.dma_start(bias_h[:], bias_h_d[:])
                ones = constp.tile([1, 128], BF16, name="ones_sb")
                nc.sync.dma_start(ones[:], ones_d[:])

            def load_xT(s):
                xT_t = xsp.tile([128, KC, 128], BF16, name="xT_t", tag="xT_t", bufs=5)
                nc.sync.dma_start(xT_t[:], xT_d[s])
                return xT_t

            def load_xr(s):
                xr_t = xsp.tile([128, F], BF16, name="xr_t", tag="xr_t", bufs=4)
                nc.sync.dma_start(xr_t[:], xr_d[s])
                return xr_t

            def new_gi_psums():
                r_ps = psp.tile([128, F], F32, name="r_ps", tag="r_ps", bufs=2)
                z_ps = psp.tile([128, F], F32, name="z_ps", tag="z_ps", bufs=2)
                inn_ps = psp.tile([128, F], F32, name="inn_ps", tag="inn_ps", bufs=2)
                return r_ps, z_ps, inn_ps

            def gi_gate(xT_t, dst, gate, kcs, final):
                """Emit gi matmuls for one gate over the given kc chunks."""
                lo = gate * F
                for kc in kcs:
                    nc.tensor.matmul(
                        dst[:], xT_t[:, kc, :], wih[:, kc, lo:lo + F],
                        start=(kc == 0),
                        stop=final and (kc == KC - 1) and not has_bias)
                if final and has_bias and kcs[-1] == KC - 1:
                    nc.tensor.matmul(dst[:], ones[:], bias_i[:, lo:lo + F],
                                     start=False, stop=final)

            def step_pe(h2, r_ps, z_ps, nxt_xT, last):
                """Recurrent matmuls for this step with next-step gi matmuls
                interleaved as fill ahead of each dependent gh group."""
                tr_ps = psp.tile([128, KC, 128], F32, name="tr_ps", tag="tr_ps", bufs=1)
                hT_t = ewp.tile([128, KC, 128], BF16, name="hT_t", tag="hT_t", bufs=2)
                hn_ps = psp.tile([128, F], F32, name="hn_ps", tag="hn_ps", bufs=1)
                nxt = new_gi_psums() if nxt_xT is not None else None

                def tr(kc):
                    nc.tensor.matmul(
                        tr_ps[:, kc, :], h2[:, kc * 128:(kc + 1) * 128], ident[:],
                        is_transpose=True, start=(kc == 0), stop=(kc == KC - 1))
                    nc.vector.tensor_copy(hT_t[:, kc, :], tr_ps[:, kc, :])

                def mm(dst, kc, lo, n, start, stop):
                    nc.tensor.matmul(
                        dst, hT_t[:, kc, :], whh[:, kc, lo:lo + n],
                        start=start, stop=stop and not has_bias)

                tr(0); tr(1)
                if nxt:
                    gi_gate(nxt_xT, nxt[0], 0, (0, 1), final=False)
                mm(r_ps[:], 0, 0, F, False, False)
                mm(r_ps[:], 1, 0, F, False, False)
                mm(hn_ps[:], 0, 2 * F, F, True, False)
                mm(hn_ps[:], 1, 2 * F, F, False, False)
                tr(2); tr(3)
                if nxt:
                    gi_gate(nxt_xT, nxt[0], 0, (2, 3), final=False)
                mm(r_ps[:], 2, 0, F, False, False)
                mm(r_ps[:], 3, 0, F, False, True)
                mm(hn_ps[:], 2, 2 * F, F, False, False)
                mm(hn_ps[:], 3, 2 * F, F, False, True)
                for kc in range(KC):
                    mm(z_ps[:], kc, F, F, False, kc == KC - 1)
                if nxt:
                    gi_gate(nxt_xT, nxt[1], 1, (0, 1, 2, 3), final=False)
                    gi_gate(nxt_xT, nxt[2], 2, (0, 1, 2, 3), final=False)
                if has_bias:
                    nc.tensor.matmul(r_ps[:], ones[:], bias_h[:, 0:F],
                                     start=False, stop=True)
                    nc.tensor.matmul(z_ps[:], ones[:], bias_h[:, F:2 * F],
                                     start=False, stop=True)
                    for half in range(2):
                        lo = 2 * F + half * H
                        nc.tensor.matmul(
                            hn_ps[:, half * H:(half + 1) * H], ones[:],
                            bias_h[:, lo:lo + H], start=False, stop=True)
                return hn_ps, nxt

            # ---- main loop ----
            xT_tiles = {0: load_xT(0), 1: load_xT(1)}
            xr_t = load_xr(0)
            for gate in range(1, 3):
                nc.sync.dma_start(wih[:, :, gate * F:(gate + 1) * F],
                                  wih_d[:, :, gate * F:(gate + 1) * F])
            nc.sync.dma_start(whh[:], whh_d[:])
            nc.sync.dma_start(ident[:], ident_d[:])
            xT_tiles[2] = load_xT(2)
            xT_tiles[3] = load_xT(3)
            xr_tiles = {1: load_xr(1), 2: load_xr(2)}
            cur = new_gi_psums()
            for gate in range(3):
                gi_gate(xT_tiles[0], cur[gate], gate, (0, 1, 2, 3), final=True)
            h2_prev = None
            for s in range(S):
                r_ps, z_ps, inn_ps = cur
                nxt_xT = xT_tiles[s + 1] if s + 1 < S else None
                if s > 0:
                    hn_ps, nxt = step_pe(h2_prev, r_ps, z_ps, nxt_xT, s == S - 1)
                elif nxt_xT is not None:
                    nxt = new_gi_psums()
                    for gate in range(3):
                        gi_gate(nxt_xT, nxt[gate], gate, (0, 1, 2, 3),
                                final=False)

                r_s = ewp.tile([128, F], BF16, name="r_s", tag="r_s", bufs=2)
                nc.scalar.activation(r_s[:, 0:H], r_ps[:, 0:H], ACT.Sigmoid)
                nc.scalar.activation(r_s[:, H:F], r_ps[:, H:F], ACT.Sigmoid)
                z_s = ewp.tile([128, F], F32, name="z_s", tag="z_s", bufs=2)
                nc.scalar.activation(z_s[:], z_ps[:], ACT.Sigmoid)

                # u = 1-z = sigmoid(-z_pre) on ACT (light queue); zh/q halves
                # run on DVE inside the tanh shadows so rhn/npre stay at the
                # DVE queue front (npre also releases the gi-fill psum WAR).
                # read z_s (SBUF), not z_ps: keeps the z_ps WAR release at
                # the z sigmoid so next-step gi_z fill is never head-blocked
                u_s = ewp.tile([128, F], BF16, name="u_s", tag="u_s", bufs=2)
                nc.scalar.activation(u_s[:], z_s[:], ACT.Copy,
                                     scale=-1.0, bias=1.0)

                # n chain + h2 (f32), halved along features so the next step's
                # transposes/matmuls start on half 0 while half 1 finishes
                h2 = ewp.tile([128, F], F32, name="h2", tag="h2", bufs=3)
                for hh in range(2):
                    sl = slice(hh * H, (hh + 1) * H)
                    if s > 0:
                        rhn = ewp.tile([128, H], BF16, name="rhn", tag="rhn", bufs=3)
                        nc.vector.tensor_mul(rhn[:], r_s[:, sl], hn_ps[:, sl])
                        npre = ewp.tile([128, H], BF16, name="npre", tag="npre", bufs=3)
                        nc.vector.tensor_add(npre[:], rhn[:], inn_ps[:, sl])
                        n_in = npre[:]
                    else:
                        n_in = inn_ps[:, sl]
                    q_h = None
                    if s > 0:
                        zh = ewp.tile([128, H], F32, name="zh", tag="zh", bufs=2)
                        nc.vector.tensor_mul(zh[:], z_s[:, sl], h2_prev[:, sl])
                        q_h = ewp.tile([128, H], F32, name="q_h", tag="q_h", bufs=3)
                        nc.vector.tensor_add(q_h[:], zh[:], xr_t[:, sl])
                    n_s = ewp.tile([128, H], BF16, name="n_s", tag="n_s", bufs=3)
                    nc.scalar.activation(n_s[:], n_in, ACT.Tanh)
                    un = ewp.tile([128, H], BF16, name="un", tag="un", bufs=3)
                    nc.vector.tensor_mul(un[:], u_s[:, sl], n_s[:])
                    # h2 written in quarters: each unblocks its transpose
                    for qq in range(2):
                        qsl = slice(hh * H + qq * 128, hh * H + (qq + 1) * 128)
                        usl = slice(qq * 128, (qq + 1) * 128)
                        src2 = q_h[:, usl] if q_h is not None else xr_t[:, qsl]
                        nc.vector.tensor_add(h2[:, qsl], un[:, usl], src2)

                if s + 1 < S:
                    cur = nxt
                if s + 3 < S and (s + 3) not in xr_tiles:
                    xr_tiles[s + 3] = load_xr(s + 3)
                if s + 4 < S and (s + 4) not in xT_tiles:
                    xT_tiles[s + 4] = load_xT(s + 4)
                xT_tiles.pop(s, None)

                if s >= WARM:
                    nc.sync.dma_start(out_d[s - WARM], h2[:])
                h2_prev = h2
                if s + 1 < S:
                    xr_t = xr_tiles.pop(s + 1)

    nc.compile()
    return nc


def _prep_core_inputs(cx, Wih, Whh, bih, bhh, core):
    """Build the per-core input map. cx: [B, T, F] fp32."""
    fwd = core < N_FWD
    k = core if fwd else core - N_FWD
    c = np.arange(NCH)
    g = NCH * k + c                                   # global chunk ids
    s = np.arange(S)
    if fwd:
        t_idx = (CHUNK * g[:, None] - WARM) + s[None, :]       # [NCH, S]
    else:
        tau = (CHUNK * g[:, None] - WARM) + s[None, :]
        t_idx = (T - 1) - tau
    valid = (t_idx >= 0) & (t_idx < T)
    t_safe = np.clip(t_idx, 0, T - 1)
    # xc[b, c, s, f]
    xc = cx[:, t_safe, :]                              # [B, NCH, S, F]
    xc = xc * valid[None, :, :, None]
    xr = np.ascontiguousarray(
        xc.transpose(2, 1, 0, 3).reshape(S, R, F), np.float32)  # [S, c*16+b, F]
    xT = np.ascontiguousarray(
        xr.reshape(S, R, KC, 128).transpose(0, 3, 2, 1))        # [S, p2, kc, r]
    Wt = np.ascontiguousarray(Wih.T.reshape(KC, 128, 3 * F).transpose(1, 0, 2))
    Ht = np.ascontiguousarray(Whh.T.reshape(KC, 128, 3 * F).transpose(1, 0, 2))
    m = {
        "xT": xT.astype(NP_BF16),
        "xr": xr.astype(NP_BF16),
        "wih": Wt.astype(NP_BF16),
        "whh": Ht.astype(NP_BF16),
        "ident": np.eye(128, dtype=np.float32),
    }
    if bih is not None:
        m["bias_i"] = bih.reshape(1, 3 * F).astype(NP_BF16)
        m["bias_h"] = bhh.reshape(1, 3 * F).astype(NP_BF16)
        m["ones"] = np.ones((1, 128), NP_BF16)
    return m


def _install_ntff_hook():
    """The agent image's antenv lacks axon_hooks; recreate it so
    run_bass_kernel_spmd(trace=True) can capture NTFF profiles."""
    import sys as _sys
    if "antenv.axon_hooks" in _sys.modules:
        return True
    so_path = "/opt/axon/libaxon_pjrt.so"
    if not os.path.exists(so_path):
        return False
    import contextlib
    import ctypes
    import types
    lib = ctypes.CDLL(so_path)
    if not hasattr(lib, "axon_start_nrt_profile"):
        return False
    lib.axon_start_nrt_profile.argtypes = [
        ctypes.POINTER(ctypes.c_int64), ctypes.c_size_t]
    lib.axon_start_nrt_profile.restype = ctypes.c_int64
    lib.axon_stop_nrt_profile.argtypes = [ctypes.c_char_p]
    lib.axon_stop_nrt_profile.restype = ctypes.c_int64

    @contextlib.contextmanager
    def _hook(output_dir, device_ids):
        import jax
        jax.devices()
        if device_ids:
            ids = (ctypes.c_int64 * len(device_ids))(*device_ids)
            rc = lib.axon_start_nrt_profile(ids, len(device_ids))
        else:
            rc = lib.axon_start_nrt_profile(None, 0)
        if rc != 0:
            raise RuntimeError(f"axon_start_nrt_profile rc={rc}")
        try:
            yield
        finally:
            n = lib.axon_stop_nrt_profile(str(output_dir).encode())
            print(f"profile: {n} file(s) written to {output_dir}",
                  file=sys.stderr)

    mod = types.ModuleType("antenv.axon_hooks")
    mod.get_axon_ntff_profile_hook = lambda: _hook
    mod.set_axon_ntff_profile_hook = lambda h: None
    _sys.modules["antenv.axon_hooks"] = mod
    return True


def _run(inputs, trace=False):
    input_x = np.asarray(inputs["input_x"], np.float32)
    Wih_f = np.asarray(inputs["Wih_f"], np.float32)
    Whh_f = np.asarray(inputs["Whh_f"], np.float32)
    Wih_b = np.asarray(inputs["Wih_b"], np.float32)
    Whh_b = np.asarray(inputs["Whh_b"], np.float32)
    bih_f = np.asarray(inputs["bih_f"], np.float32)
    bhh_f = np.asarray(inputs["bhh_f"], np.float32)
    bih_b = np.asarray(inputs["bih_b"], np.float32)
    bhh_b = np.asarray(inputs["bhh_b"], np.float32)
    L = int(inputs["L"])

    has_bias = bool(
        np.any(bih_f) or np.any(bhh_f) or np.any(bih_b) or np.any(bhh_b))
    key = (has_bias, S, CHUNK)
    if key not in _PROG_CACHE:
        _PROG_CACHE[key] = _build_program(has_bias)
    nc = _PROG_CACHE[key]

    cx = np.ascontiguousarray(input_x[:, :, :F])
    in_maps = []
    for core in range(N_CORES):
        fwd = core < N_FWD
        in_maps.append(_prep_core_inputs(
            cx,
            Wih_f if fwd else Wih_b,
            Whh_f if fwd else Whh_b,
            (bih_f if fwd else bih_b) if has_bias else None,
            (bhh_f if fwd else bhh_b) if has_bias else None,
            core,
        ))

    if trace and not _install_ntff_hook():
        trace = False
    res = run_bass_kernel_spmd(nc, in_maps, list(range(N_CORES)), trace=trace)

    # reassemble: hs[dir][b, t, F]
    hs_f = np.empty((B, T, F), np.float32)
    hs_b = np.empty((B, T, F), np.float32)
    for core in range(N_CORES):
        o = np.asarray(res.results[core]["out"]).astype(np.float32)
        o = o.reshape(CHUNK, NCH, B, F)
        o = o.transpose(1, 2, 0, 3)                    # [c, b, chunk, F]
        fwd = core < N_FWD
        k = core if fwd else core - N_FWD
        dst = hs_f if fwd else hs_b
        for c in range(NCH):
            t0 = CHUNK * (NCH * k + c)
            dst[:, t0:t0 + CHUNK, :] = o[c]
    out = np.empty((B, T - 2 * L, 2 * F), np.float32)
    out[:, :, :F] = hs_f[:, L:T - L, :]
    out[:, :, F:] = hs_b[:, L:T - L, :]
    return out, res


def kernel(**inputs) -> np.ndarray:
    out, _ = _run(inputs, trace=False)
    return out


# revision 24
# speedup vs baseline: 1.2192x; 1.2192x over previous
"""BiGRU encoder on 8 Trainium2 NeuronCores.

Strategy: the T=2048 recurrence is split into 32 chunks per direction of 64
steps each, computed in parallel as independent chains with a 32-step warm-up
prefix (the GRU state's dependence on its past decays geometrically; W=32
gives ~5e-3 relative error vs an exact scan). Cores 0-3 run the forward
direction (8 chains x 16 batch = 128 rows each), cores 4-7 the backward
direction on host-reversed data. Per step, each core does:
  gates = [x_t | h_{t-1}] @ [Wih | Whh]^T  as bf16 matmuls (stationary = xT /
  hT chunks of 128 rows, moving = weight tiles [128,512]), accumulated in
  f32 PSUM; sigmoid/tanh on ACT (bf16 gates); GRU update on DVE with the
  state h kept in f32 (bf16 state lets quantization noise compound through
  the residual stream). h is transposed for the next step's matmul with
  PE-transpose; the psum->sbuf hT copies run on DVE (casting to bf16) so the
  ACT queue stays short. Next-step gi matmuls are interleaved into the
  recurrent matmul sequence as fill so the PE never stalls at the head of
  its queue.
The host slices x, builds the per-core layouts, and reassembles the output.
"""
import os
import sys
import numpy as np
import ml_dtypes

try:
    import concourse.bass as bass
except ImportError:
    import sys
    sys.path.insert(0, "/opt/trn_rl_repo")
    import concourse.bass as bass

import concourse.tile as tile
from concourse import bacc, mybir
from concourse.bass_utils import run_bass_kernel_spmd

F32 = mybir.dt.float32
BF16 = mybir.dt.bfloat16
NP_BF16 = ml_dtypes.bfloat16

# geometry (hardcoded for this problem)
B = 16          # batch
T = 2048        # timesteps
F = 512         # hidden/feature size
H = F // 2
KC = 4          # contraction chunks (F / 128)
CHUNK = int(os.environ.get("GRU_CHUNK", "64"))   # stored steps per chain
WARM = int(os.environ.get("GRU_WARM", "30"))     # warm-up steps per chain
S = CHUNK + WARM                                  # total steps per core
NCH = 8         # chains per core
R = NCH * B     # rows per core = 128
N_CORES = 8
N_FWD = 4       # cores 0..3 forward, 4..7 backward
ACT = mybir.ActivationFunctionType
ALU = mybir.AluOpType

_PROG_CACHE = {}


def _build_program(has_bias: bool):
    nc = bacc.Bacc("TRN2", target_bir_lowering=False, debug=False)

    xT_d = nc.dram_tensor("xT", [S, 128, KC, 128], BF16, kind="ExternalInput").ap()
    xr_d = nc.dram_tensor("xr", [S, 128, F], BF16, kind="ExternalInput").ap()
    wih_d = nc.dram_tensor("wih", [128, KC, 3 * F], BF16, kind="ExternalInput").ap()
    whh_d = nc.dram_tensor("whh", [128, KC, 3 * F], BF16, kind="ExternalInput").ap()
    ident_d = nc.dram_tensor("ident", [128, 128], F32, kind="ExternalInput").ap()
    if has_bias:
        bias_i_d = nc.dram_tensor("bias_i", [1, 3 * F], BF16, kind="ExternalInput").ap()
        bias_h_d = nc.dram_tensor("bias_h", [1, 3 * F], BF16, kind="ExternalInput").ap()
        ones_d = nc.dram_tensor("ones", [1, 128], BF16, kind="ExternalInput").ap()
    out_d = nc.dram_tensor("out", [CHUNK, 128, F], F32, kind="ExternalOutput").ap()

    with tile.TileContext(nc) as tc:
        with (
            tc.tile_pool(name="const", bufs=1) as constp,
            tc.tile_pool(name="xs", bufs=1) as xsp,
            tc.tile_pool(name="ew", bufs=1) as ewp,
            tc.tile_pool(name="ps", bufs=1, space="PSUM") as psp,
        ):
            # load wih gate-by-gate so the first gi matmuls can start as
            # soon as the r-slice + xT(0) land rather than after the full 3MB
            wih = constp.tile([128, KC, 3 * F], BF16, name="wih_sb")
            whh = constp.tile([128, KC, 3 * F], BF16, name="whh_sb")
            ident = constp.tile([128, 128], F32, name="ident_sb")
            nc.sync.dma_start(wih[:, :, 0:F], wih_d[:, :, 0:F])
            if has_bias:
                bias_i = constp.tile([1, 3 * F], BF16, name="bias_i_sb")
                nc.sync.dma_start(bias_i[:], bias_i_d[:])
                bias_h = constp.tile([1, 3 * F], BF16, name="bias_h_sb")
                nc.sync.dma_start(bias_h[:], bias_h_d[:])
                ones = constp.tile([1, 128], BF16, name="ones_sb")
                nc.sync.dma_start(ones[:], ones_d[:])

            def load_xT(s):
                xT_t = xsp.tile([128, KC, 128], BF16, name="xT_t", tag="xT_t", bufs=5)
                nc.sync.dma_start(xT_t[:], xT_d[s])
                return xT_t

            def load_xr(s):
                xr_t = xsp.tile([128, F], BF16, name="xr_t", tag="xr_t", bufs=4)
                nc.sync.dma_start(xr_t[:], xr_d[s])
                return xr_t

            def new_gi_psums():
                r_ps = psp.tile([128, F], F32, name="r_ps", tag="r_ps", bufs=2)
                z_ps = psp.tile([128, F], F32, name="z_ps", tag="z_ps", bufs=2)
                inn_ps = psp.tile([128, F], F32, name="inn_ps", tag="inn_ps", bufs=2)
                return r_ps, z_ps, inn_ps

            def gi_gate(xT_t, dst, gate, kcs, final):
                """Emit gi matmuls for one gate over the given kc chunks."""
                lo = gate * F
                for kc in kcs:
                    nc.tensor.matmul(
                        dst[:], xT_t[:, kc, :], wih[:, kc, lo:lo + F],
                        start=(kc == 0),
                        stop=final and (kc == KC - 1) and not has_bias)
                if final and has_bias and kcs[-1] == KC - 1:
                    nc.tensor.matmul(dst[:], ones[:], bias_i[:, lo:lo + F],
                                     start=False, stop=final)

            def step_pe(h2, r_ps, z_ps, nxt_xT, last):
                """Recurrent matmuls for this step with next-step gi matmuls
                interleaved as fill ahead of each dependent gh group."""
                tr_ps = psp.tile([128, KC, 128], F32, name="tr_ps", tag="tr_ps", bufs=1)
                hT_t = ewp.tile([128, KC, 128], BF16, name="hT_t", tag="hT_t", bufs=2)
                hn_ps = psp.tile([128, F], F32, name="hn_ps", tag="hn_ps", bufs=1)
                nxt = new_gi_psums() if nxt_xT is not None else None

                def tr(kc):
                    nc.tensor.matmul(
                        tr_ps[:, kc, :], h2[:, kc * 128:(kc + 1) * 128], ident[:],
                        is_transpose=True, start=(kc == 0), stop=(kc == KC - 1))
                    nc.scalar.copy(hT_t[:, kc, :], tr_ps[:, kc, :])

                def mm(dst, kc, lo, n, start, stop):
                    nc.tensor.matmul(
                        dst, hT_t[:, kc, :], whh[:, kc, lo:lo + n],
                        start=start, stop=stop and not has_bias)

                tr(0); tr(1)
                if nxt:
                    gi_gate(nxt_xT, nxt[0], 0, (0, 1), final=False)
                mm(r_ps[:], 0, 0, F, False, False)
                mm(r_ps[:], 1, 0, F, False, False)
                mm(hn_ps[:, 0:H], 0, 2 * F, H, True, False)
                mm(hn_ps[:, 0:H], 1, 2 * F, H, False, False)
                tr(2); tr(3)
                if nxt:
                    gi_gate(nxt_xT, nxt[0], 0, (2, 3), final=False)
                mm(r_ps[:], 2, 0, F, False, False)
                mm(r_ps[:], 3, 0, F, False, True)
                mm(hn_ps[:, 0:H], 2, 2 * F, H, False, False)
                mm(hn_ps[:, 0:H], 3, 2 * F, H, False, True)
                for kc in range(KC):
                    mm(z_ps[:], kc, F, F, False, kc == KC - 1)
                for kc in range(KC):
                    mm(hn_ps[:, H:F], kc, 2 * F + H, H, False, kc == KC - 1)
                if nxt:
                    gi_gate(nxt_xT, nxt[1], 1, (0, 1, 2, 3), final=False)
                    gi_gate(nxt_xT, nxt[2], 2, (0, 1, 2, 3), final=False)
                if has_bias:
                    nc.tensor.matmul(r_ps[:], ones[:], bias_h[:, 0:F],
                                     start=False, stop=True)
                    nc.tensor.matmul(z_ps[:], ones[:], bias_h[:, F:2 * F],
                                     start=False, stop=True)
                    for half in range(2):
                        lo = 2 * F + half * H
                        nc.tensor.matmul(
                            hn_ps[:, half * H:(half + 1) * H], ones[:],
                            bias_h[:, lo:lo + H], start=False, stop=True)
                return hn_ps, nxt

            # ---- main loop ----
            xT_tiles = {0: load_xT(0), 1: load_xT(1)}
            xr_t = load_xr(0)
            for gate in range(1, 3):
                nc.sync.dma_start(wih[:, :, gate * F:(gate + 1) * F],
                                  wih_d[:, :, gate * F:(gate + 1) * F])
            nc.sync.dma_start(whh[:], whh_d[:])
            nc.sync.dma_start(ident[:], ident_d[:])
            xT_tiles[2] = load_xT(2)
            xT_tiles[3] = load_xT(3)
            xr_tiles = {1: load_xr(1), 2: load_xr(2)}
            cur = new_gi_psums()
            for gate in range(3):
                gi_gate(xT_tiles[0], cur[gate], gate, (0, 1, 2, 3), final=True)
            h2_prev = None
            for s in range(S):
                r_ps, z_ps, inn_ps = cur
                nxt_xT = xT_tiles[s + 1] if s + 1 < S else None
                if s > 0:
                    hn_ps, nxt = step_pe(h2_prev, r_ps, z_ps, nxt_xT, s == S - 1)
                elif nxt_xT is not None:
                    nxt = new_gi_psums()
                    for gate in range(3):
                        gi_gate(nxt_xT, nxt[gate], gate, (0, 1, 2, 3),
                                final=False)

                r_s = ewp.tile([128, F], BF16, name="r_s", tag="r_s", bufs=2)
                nc.scalar.activation(r_s[:, 0:H], r_ps[:, 0:H], ACT.Sigmoid)
                nc.scalar.activation(r_s[:, H:F], r_ps[:, H:F], ACT.Sigmoid)
                z_s = ewp.tile([128, F], F32, name="z_s", tag="z_s", bufs=2)
                nc.scalar.activation(z_s[:], z_ps[:], ACT.Sigmoid)

                # u = 1-z = sigmoid(-z_pre) on ACT (light queue); zh/q halves
                # run on DVE inside the tanh shadows so rhn/npre stay at the
                # DVE queue front (npre also releases the gi-fill psum WAR).
                # read z_s (SBUF), not z_ps: keeps the z_ps WAR release at
                # the z sigmoid so next-step gi_z fill is never head-blocked
                u_s = ewp.tile([128, F], BF16, name="u_s", tag="u_s", bufs=2)
                nc.scalar.activation(u_s[:], z_s[:], ACT.Copy,
                                     scale=-1.0, bias=1.0)

                # n chain + h2 (f32), halved along features so the next step's
                # transposes/matmuls start on half 0 while half 1 finishes
                h2 = ewp.tile([128, F], F32, name="h2", tag="h2", bufs=3)
                for hh in range(2):
                    sl = slice(hh * H, (hh + 1) * H)
                    if s > 0:
                        rhn = ewp.tile([128, H], BF16, name="rhn", tag="rhn", bufs=3)
                        nc.vector.tensor_mul(rhn[:], r_s[:, sl], hn_ps[:, sl])
                        npre = ewp.tile([128, H], BF16, name="npre", tag="npre", bufs=3)
                        nc.vector.tensor_add(npre[:], rhn[:], inn_ps[:, sl])
                        n_in = npre[:]
                    else:
                        n_in = inn_ps[:, sl]
                    q_h = None
                    if s > 0:
                        zh = ewp.tile([128, H], F32, name="zh", tag="zh", bufs=2)
                        nc.vector.tensor_mul(zh[:], z_s[:, sl], h2_prev[:, sl])
                        q_h = ewp.tile([128, H], F32, name="q_h", tag="q_h", bufs=3)
                        nc.vector.tensor_add(q_h[:], zh[:], xr_t[:, sl])
                    n_s = ewp.tile([128, H], BF16, name="n_s", tag="n_s", bufs=3)
                    nc.scalar.activation(n_s[:], n_in, ACT.Tanh)
                    un = ewp.tile([128, H], BF16, name="un", tag="un", bufs=3)
                    nc.vector.tensor_mul(un[:], u_s[:, sl], n_s[:])
                    # h2 written in quarters: each unblocks its transpose
                    for qq in range(2):
                        qsl = slice(hh * H + qq * 128, hh * H + (qq + 1) * 128)
                        usl = slice(qq * 128, (qq + 1) * 128)
                        src2 = q_h[:, usl] if q_h is not None else xr_t[:, qsl]
                        nc.vector.tensor_add(h2[:, qsl], un[:, usl], src2)

                if s + 1 < S:
                    cur = nxt
                if s + 3 < S and (s + 3) not in xr_tiles:
                    xr_tiles[s + 3] = load_xr(s + 3)
                if s + 4 < S and (s + 4) not in xT_tiles:
                    xT_tiles[s + 4] = load_xT(s + 4)
                xT_tiles.pop(s, None)

                if s >= WARM:
                    nc.sync.dma_start(out_d[s - WARM], h2[:])
                h2_prev = h2
                if s + 1 < S:
                    xr_t = xr_tiles.pop(s + 1)

    nc.compile()
    return nc


def _prep_core_inputs(cx, Wih, Whh, bih, bhh, core):
    """Build the per-core input map. cx: [B, T, F] fp32."""
    fwd = core < N_FWD
    k = core if fwd else core - N_FWD
    c = np.arange(NCH)
    g = NCH * k + c                                   # global chunk ids
    s = np.arange(S)
    if fwd:
        t_idx = (CHUNK * g[:, None] - WARM) + s[None, :]       # [NCH, S]
    else:
        tau = (CHUNK * g[:, None] - WARM) + s[None, :]
        t_idx = (T - 1) - tau
    valid = (t_idx >= 0) & (t_idx < T)
    t_safe = np.clip(t_idx, 0, T - 1)
    # xc[b, c, s, f]
    xc = cx[:, t_safe, :]                              # [B, NCH, S, F]
    xc = xc * valid[None, :, :, None]
    xr = np.ascontiguousarray(
        xc.transpose(2, 1, 0, 3).reshape(S, R, F), np.float32)  # [S, c*16+b, F]
    xT = np.ascontiguousarray(
        xr.reshape(S, R, KC, 128).transpose(0, 3, 2, 1))        # [S, p2, kc, r]
    Wt = np.ascontiguousarray(Wih.T.reshape(KC, 128, 3 * F).transpose(1, 0, 2))
    Ht = np.ascontiguousarray(Whh.T.reshape(KC, 128, 3 * F).transpose(1, 0, 2))
    m = {
        "xT": xT.astype(NP_BF16),
        "xr": xr.astype(NP_BF16),
        "wih": Wt.astype(NP_BF16),
        "whh": Ht.astype(NP_BF16),
        "ident": np.eye(128, dtype=np.float32),
    }
    if bih is not None:
        m["bias_i"] = bih.reshape(1, 3 * F).astype(NP_BF16)
        m["bias_h"] = bhh.reshape(1, 3 * F).astype(NP_BF16)
        m["ones"] = np.ones((1, 128), NP_BF16)
    return m


def _install_ntff_hook():
    """The agent image's antenv lacks axon_hooks; recreate it so
    run_bass_kernel_spmd(trace=True) can capture NTFF profiles."""
    import sys as _sys
    if "antenv.axon_hooks" in _sys.modules:
        return True
    so_path = "/opt/axon/libaxon_pjrt.so"
    if not os.path.exists(so_path):
        return False
    import contextlib
    import ctypes
    import types
    lib = ctypes.CDLL(so_path)
    if not hasattr(lib, "axon_start_nrt_profile"):
        return False
    lib.axon_start_nrt_profile.argtypes = [
        ctypes.POINTER(ctypes.c_int64), ctypes.c_size_t]
    lib.axon_start_nrt_profile.restype = ctypes.c_int64
    lib.axon_stop_nrt_profile.argtypes = [ctypes.c_char_p]
    lib.axon_stop_nrt_profile.restype = ctypes.c_int64

    @contextlib.contextmanager
    def _hook(output_dir, device_ids):
        import jax
        jax.devices()
        if device_ids:
            ids = (ctypes.c_int64 * len(device_ids))(*device_ids)
            rc = lib.axon_start_nrt_profile(ids, len(device_ids))
        else:
            rc = lib.axon_start_nrt_profile(None, 0)
        if rc != 0:
            raise RuntimeError(f"axon_start_nrt_profile rc={rc}")
        try:
            yield
        finally:
            n = lib.axon_stop_nrt_profile(str(output_dir).encode())
            print(f"profile: {n} file(s) written to {output_dir}",
                  file=sys.stderr)

    mod = types.ModuleType("antenv.axon_hooks")
    mod.get_axon_ntff_profile_hook = lambda: _hook
    mod.set_axon_ntff_profile_hook = lambda h: None
    _sys.modules["antenv.axon_hooks"] = mod
    return True


def _run(inputs, trace=False):
    input_x = np.asarray(inputs["input_x"], np.float32)
    Wih_f = np.asarray(inputs["Wih_f"], np.float32)
    Whh_f = np.asarray(inputs["Whh_f"], np.float32)
    Wih_b = np.asarray(inputs["Wih_b"], np.float32)
    Whh_b = np.asarray(inputs["Whh_b"], np.float32)
    bih_f = np.asarray(inputs["bih_f"], np.float32)
    bhh_f = np.asarray(inputs["bhh_f"], np.float32)
    bih_b = np.asarray(inputs["bih_b"], np.float32)
    bhh_b = np.asarray(inputs["bhh_b"], np.float32)
    L = int(inputs["L"])

    has_bias = bool(
        np.any(bih_f) or np.any(bhh_f) or np.any(bih_b) or np.any(bhh_b))
    key = (has_bias, S, CHUNK)
    if key not in _PROG_CACHE:
        _PROG_CACHE[key] = _build_program(has_bias)
    nc = _PROG_CACHE[key]

    cx = np.ascontiguousarray(input_x[:, :, :F])
    in_maps = []
    for core in range(N_CORES):
        fwd = core < N_FWD
        in_maps.append(_prep_core_inputs(
            cx,
            Wih_f if fwd else Wih_b,
            Whh_f if fwd else Whh_b,
            (bih_f if fwd else bih_b) if has_bias else None,
            (bhh_f if fwd else bhh_b) if has_bias else None,
            core,
        ))

    if trace and not _install_ntff_hook():
        trace = False
    res = run_bass_kernel_spmd(nc, in_maps, list(range(N_CORES)), trace=trace)

    # reassemble: hs[dir][b, t, F]
    hs_f = np.empty((B, T, F), np.float32)
    hs_b = np.empty((B, T, F), np.float32)
    for core in range(N_CORES):
        o = np.asarray(res.results[core]["out"]).astype(np.float32)
        o = o.reshape(CHUNK, NCH, B, F)
        o = o.transpose(1, 2, 0, 3)                    # [c, b, chunk, F]
        fwd = core < N_FWD
        k = core if fwd else core - N_FWD
        dst = hs_f if fwd else hs_b
        for c in range(NCH):
            t0 = CHUNK * (NCH * k + c)
            dst[:, t0:t0 + CHUNK, :] = o[c]
    out = np.empty((B, T - 2 * L, 2 * F), np.float32)
    out[:, :, :F] = hs_f[:, L:T - L, :]
    out[:, :, F:] = hs_b[:, L:T - L, :]
    return out, res


def kernel(**inputs) -> np.ndarray:
    out, _ = _run(inputs, trace=False)
    return out


# revision 26
# speedup vs baseline: 1.2316x; 1.0102x over previous
"""BiGRU encoder on 8 Trainium2 NeuronCores.

Strategy: the T=2048 recurrence is split into 32 chunks per direction of 64
steps each, computed in parallel as independent chains with a 32-step warm-up
prefix (the GRU state's dependence on its past decays geometrically; W=32
gives ~5e-3 relative error vs an exact scan). Cores 0-3 run the forward
direction (8 chains x 16 batch = 128 rows each), cores 4-7 the backward
direction on host-reversed data. Per step, each core does:
  gates = [x_t | h_{t-1}] @ [Wih | Whh]^T  as bf16 matmuls (stationary = xT /
  hT chunks of 128 rows, moving = weight tiles [128,512]), accumulated in
  f32 PSUM; sigmoid/tanh on ACT (bf16 gates); GRU update on DVE with the
  state h kept in f32 (bf16 state lets quantization noise compound through
  the residual stream). h is transposed for the next step's matmul with
  PE-transpose; the psum->sbuf hT copies run on DVE (casting to bf16) so the
  ACT queue stays short. Next-step gi matmuls are interleaved into the
  recurrent matmul sequence as fill so the PE never stalls at the head of
  its queue.
The host slices x, builds the per-core layouts, and reassembles the output.
"""
import os
import sys
import numpy as np
import ml_dtypes

try:
    import concourse.bass as bass
except ImportError:
    import sys
    sys.path.insert(0, "/opt/trn_rl_repo")
    import concourse.bass as bass

import concourse.tile as tile
from concourse import bacc, mybir
from concourse.bass_utils import run_bass_kernel_spmd

F32 = mybir.dt.float32
BF16 = mybir.dt.bfloat16
NP_BF16 = ml_dtypes.bfloat16

# geometry (hardcoded for this problem)
B = 16          # batch
T = 2048        # timesteps
F = 512         # hidden/feature size
H = F // 2
KC = 4          # contraction chunks (F / 128)
CHUNK = int(os.environ.get("GRU_CHUNK", "64"))   # stored steps per chain
WARM = int(os.environ.get("GRU_WARM", "30"))     # warm-up steps per chain
S = CHUNK + WARM                                  # total steps per core
NCH = 8         # chains per core
R = NCH * B     # rows per core = 128
N_CORES = 8
N_FWD = 4       # cores 0..3 forward, 4..7 backward
ACT = mybir.ActivationFunctionType
ALU = mybir.AluOpType

_PROG_CACHE = {}


def _build_program(has_bias: bool):
    nc = bacc.Bacc("TRN2", target_bir_lowering=False, debug=False)

    xT_d = nc.dram_tensor("xT", [S, 128, KC, 128], BF16, kind="ExternalInput").ap()
    xr_d = nc.dram_tensor("xr", [S, 128, F], BF16, kind="ExternalInput").ap()
    wih_d = nc.dram_tensor("wih", [128, KC, 3 * F], BF16, kind="ExternalInput").ap()
    whh_d = nc.dram_tensor("whh", [128, KC, 3 * F], BF16, kind="ExternalInput").ap()
    ident_d = nc.dram_tensor("ident", [128, 128], F32, kind="ExternalInput").ap()
    if has_bias:
        bias_i_d = nc.dram_tensor("bias_i", [1, 3 * F], BF16, kind="ExternalInput").ap()
        bias_h_d = nc.dram_tensor("bias_h", [1, 3 * F], BF16, kind="ExternalInput").ap()
        ones_d = nc.dram_tensor("ones", [1, 128], BF16, kind="ExternalInput").ap()
    out_d = nc.dram_tensor("out", [CHUNK, 128, F], F32, kind="ExternalOutput").ap()

    with tile.TileContext(nc) as tc:
        with (
            tc.tile_pool(name="const", bufs=1) as constp,
            tc.tile_pool(name="xs", bufs=1) as xsp,
            tc.tile_pool(name="ew", bufs=1) as ewp,
            tc.tile_pool(name="ps", bufs=1, space="PSUM") as psp,
        ):
            # load wih gate-by-gate so the first gi matmuls can start as
            # soon as the r-slice + xT(0) land rather than after the full 3MB
            wih = constp.tile([128, KC, 3 * F], BF16, name="wih_sb")
            whh = constp.tile([128, KC, 3 * F], BF16, name="whh_sb")
            ident = constp.tile([128, 128], F32, name="ident_sb")
            nc.sync.dma_start(wih[:, :, 0:F], wih_d[:, :, 0:F])
            if has_bias:
                bias_i = constp.tile([1, 3 * F], BF16, name="bias_i_sb")
                nc.sync.dma_start(bias_i[:], bias_i_d[:])
                bias_h = constp.tile([1, 3 * F], BF16, name="bias_h_sb")
                nc.sync.dma_start(bias_h[:], bias_h_d[:])
                ones = constp.tile([1, 128], BF16, name="ones_sb")
                nc.sync.dma_start(ones[:], ones_d[:])

            def load_xT(s):
                xT_t = xsp.tile([128, KC, 128], BF16, name="xT_t", tag="xT_t", bufs=5)
                nc.sync.dma_start(xT_t[:], xT_d[s])
                return xT_t

            def load_xr(s):
                xr_t = xsp.tile([128, F], BF16, name="xr_t", tag="xr_t", bufs=4)
                nc.sync.dma_start(xr_t[:], xr_d[s])
                return xr_t

            def new_gi_psums():
                r_ps = psp.tile([128, F], F32, name="r_ps", tag="r_ps", bufs=2)
                z_ps = psp.tile([128, F], F32, name="z_ps", tag="z_ps", bufs=2)
                inn_ps = psp.tile([128, F], F32, name="inn_ps", tag="inn_ps", bufs=2)
                return r_ps, z_ps, inn_ps

            def gi_gate(xT_t, dst, gate, kcs, final):
                """Emit gi matmuls for one gate over the given kc chunks."""
                lo = gate * F
                for kc in kcs:
                    nc.tensor.matmul(
                        dst[:], xT_t[:, kc, :], wih[:, kc, lo:lo + F],
                        start=(kc == 0),
                        stop=final and (kc == KC - 1) and not has_bias)
                if final and has_bias and kcs[-1] == KC - 1:
                    nc.tensor.matmul(dst[:], ones[:], bias_i[:, lo:lo + F],
                                     start=False, stop=final)

            def step_pe(h2, r_ps, z_ps, nxt_xT, last):
                """Recurrent matmuls for this step with next-step gi matmuls
                interleaved as fill ahead of each dependent gh group."""
                tr_ps = psp.tile([128, KC, 128], F32, name="tr_ps", tag="tr_ps", bufs=1)
                hT_t = ewp.tile([128, KC, 128], BF16, name="hT_t", tag="hT_t", bufs=3)
                hn_ps = psp.tile([128, F], F32, name="hn_ps", tag="hn_ps", bufs=1)
                nxt = new_gi_psums() if nxt_xT is not None else None

                def tr(kc):
                    nc.tensor.matmul(
                        tr_ps[:, kc, :], h2[:, kc * 128:(kc + 1) * 128], ident[:],
                        is_transpose=True, start=(kc == 0), stop=(kc == KC - 1))
                    # split copies across ACT/DVE so each pair lands in half
                    # the serial latency
                    if kc % 2 == 0:
                        nc.scalar.copy(hT_t[:, kc, :], tr_ps[:, kc, :])
                    else:
                        nc.vector.tensor_copy(hT_t[:, kc, :], tr_ps[:, kc, :])

                def mm(dst, kc, lo, n, start, stop):
                    nc.tensor.matmul(
                        dst, hT_t[:, kc, :], whh[:, kc, lo:lo + n],
                        start=start, stop=stop and not has_bias)

                tr(0); tr(1)
                if nxt:
                    gi_gate(nxt_xT, nxt[0], 0, (0, 1), final=False)
                mm(r_ps[:], 0, 0, F, False, False)
                mm(r_ps[:], 1, 0, F, False, False)
                mm(hn_ps[:, 0:H], 0, 2 * F, H, True, False)
                mm(hn_ps[:, 0:H], 1, 2 * F, H, False, False)
                tr(2); tr(3)
                if nxt:
                    gi_gate(nxt_xT, nxt[0], 0, (2, 3), final=False)
                mm(r_ps[:], 2, 0, F, False, False)
                mm(r_ps[:], 3, 0, F, False, True)
                mm(hn_ps[:, 0:H], 2, 2 * F, H, False, False)
                mm(hn_ps[:, 0:H], 3, 2 * F, H, False, True)
                for kc in range(KC):
                    mm(z_ps[:], kc, F, F, False, kc == KC - 1)
                for kc in range(KC):
                    mm(hn_ps[:, H:F], kc, 2 * F + H, H, False, kc == KC - 1)
                if nxt:
                    gi_gate(nxt_xT, nxt[1], 1, (0, 1, 2, 3), final=False)
                    gi_gate(nxt_xT, nxt[2], 2, (0, 1, 2, 3), final=False)
                if has_bias:
                    nc.tensor.matmul(r_ps[:], ones[:], bias_h[:, 0:F],
                                     start=False, stop=True)
                    nc.tensor.matmul(z_ps[:], ones[:], bias_h[:, F:2 * F],
                                     start=False, stop=True)
                    for half in range(2):
                        lo = 2 * F + half * H
                        nc.tensor.matmul(
                            hn_ps[:, half * H:(half + 1) * H], ones[:],
                            bias_h[:, lo:lo + H], start=False, stop=True)
                return hn_ps, nxt

            # ---- main loop ----
            xT_tiles = {0: load_xT(0), 1: load_xT(1)}
            xr_t = load_xr(0)
            for gate in range(1, 3):
                nc.sync.dma_start(wih[:, :, gate * F:(gate + 1) * F],
                                  wih_d[:, :, gate * F:(gate + 1) * F])
            nc.sync.dma_start(whh[:], whh_d[:])
            nc.sync.dma_start(ident[:], ident_d[:])
            xT_tiles[2] = load_xT(2)
            xT_tiles[3] = load_xT(3)
            xr_tiles = {1: load_xr(1), 2: load_xr(2)}
            cur = new_gi_psums()
            for gate in range(3):
                gi_gate(xT_tiles[0], cur[gate], gate, (0, 1, 2, 3), final=True)
            h2_prev = None
            for s in range(S):
                r_ps, z_ps, inn_ps = cur
                nxt_xT = xT_tiles[s + 1] if s + 1 < S else None
                if s > 0:
                    hn_ps, nxt = step_pe(h2_prev, r_ps, z_ps, nxt_xT, s == S - 1)
                elif nxt_xT is not None:
                    nxt = new_gi_psums()
                    for gate in range(3):
                        gi_gate(nxt_xT, nxt[gate], gate, (0, 1, 2, 3),
                                final=False)

                r_s = ewp.tile([128, F], BF16, name="r_s", tag="r_s", bufs=2)
                nc.scalar.activation(r_s[:, 0:H], r_ps[:, 0:H], ACT.Sigmoid)
                nc.scalar.activation(r_s[:, H:F], r_ps[:, H:F], ACT.Sigmoid)
                z_s = ewp.tile([128, F], F32, name="z_s", tag="z_s", bufs=2)
                nc.scalar.activation(z_s[:], z_ps[:], ACT.Sigmoid)

                # u = 1-z = sigmoid(-z_pre) on ACT (light queue); zh/q halves
                # run on DVE inside the tanh shadows so rhn/npre stay at the
                # DVE queue front (npre also releases the gi-fill psum WAR).
                # read z_s (SBUF), not z_ps: keeps the z_ps WAR release at
                # the z sigmoid so next-step gi_z fill is never head-blocked
                u_s = ewp.tile([128, F], BF16, name="u_s", tag="u_s", bufs=2)
                nc.scalar.activation(u_s[:], z_s[:], ACT.Copy,
                                     scale=-1.0, bias=1.0)

                # n chain + h2 (f32), halved along features so the next step's
                # transposes/matmuls start on half 0 while half 1 finishes
                h2 = ewp.tile([128, F], F32, name="h2", tag="h2", bufs=3)
                for hh in range(2):
                    sl = slice(hh * H, (hh + 1) * H)
                    if s > 0:
                        rhn = ewp.tile([128, H], BF16, name="rhn", tag="rhn", bufs=3)
                        nc.vector.tensor_mul(rhn[:], r_s[:, sl], hn_ps[:, sl])
                        npre = ewp.tile([128, H], BF16, name="npre", tag="npre", bufs=3)
                        nc.vector.tensor_add(npre[:], rhn[:], inn_ps[:, sl])
                        n_in = npre[:]
                    else:
                        n_in = inn_ps[:, sl]
                    q_h = None
                    if s > 0:
                        zh = ewp.tile([128, H], F32, name="zh", tag="zh", bufs=2)
                        nc.vector.tensor_mul(zh[:], z_s[:, sl], h2_prev[:, sl])
                        q_h = ewp.tile([128, H], F32, name="q_h", tag="q_h", bufs=3)
                        nc.vector.tensor_add(q_h[:], zh[:], xr_t[:, sl])
                    n_s = ewp.tile([128, H], BF16, name="n_s", tag="n_s", bufs=3)
                    nc.scalar.activation(n_s[:], n_in, ACT.Tanh)
                    un = ewp.tile([128, H], BF16, name="un", tag="un", bufs=3)
                    nc.vector.tensor_mul(un[:], u_s[:, sl], n_s[:])
                    # h2 written in quarters: each unblocks its transpose
                    for qq in range(2):
                        qsl = slice(hh * H + qq * 128, hh * H + (qq + 1) * 128)
                        usl = slice(qq * 128, (qq + 1) * 128)
                        src2 = q_h[:, usl] if q_h is not None else xr_t[:, qsl]
                        nc.vector.tensor_add(h2[:, qsl], un[:, usl], src2)

                if s + 1 < S:
                    cur = nxt
                if s + 3 < S and (s + 3) not in xr_tiles:
                    xr_tiles[s + 3] = load_xr(s + 3)
                if s + 4 < S and (s + 4) not in xT_tiles:
                    xT_tiles[s + 4] = load_xT(s + 4)
                xT_tiles.pop(s, None)

                if s >= WARM:
                    nc.sync.dma_start(out_d[s - WARM], h2[:])
                h2_prev = h2
                if s + 1 < S:
                    xr_t = xr_tiles.pop(s + 1)

    nc.compile()
    return nc


def _prep_core_inputs(cx, Wih, Whh, bih, bhh, core):
    """Build the per-core input map. cx: [B, T, F] fp32."""
    fwd = core < N_FWD
    k = core if fwd else core - N_FWD
    c = np.arange(NCH)
    g = NCH * k + c                                   # global chunk ids
    s = np.arange(S)
    if fwd:
        t_idx = (CHUNK * g[:, None] - WARM) + s[None, :]       # [NCH, S]
    else:
        tau = (CHUNK * g[:, None] - WARM) + s[None, :]
        t_idx = (T - 1) - tau
    valid = (t_idx >= 0) & (t_idx < T)
    t_safe = np.clip(t_idx, 0, T - 1)
    # xc[b, c, s, f]
    xc = cx[:, t_safe, :]                              # [B, NCH, S, F]
    xc = xc * valid[None, :, :, None]
    xr = np.ascontiguousarray(
        xc.transpose(2, 1, 0, 3).reshape(S, R, F), np.float32)  # [S, c*16+b, F]
    xT = np.ascontiguousarray(
        xr.reshape(S, R, KC, 128).transpose(0, 3, 2, 1))        # [S, p2, kc, r]
    Wt = np.ascontiguousarray(Wih.T.reshape(KC, 128, 3 * F).transpose(1, 0, 2))
    Ht = np.ascontiguousarray(Whh.T.reshape(KC, 128, 3 * F).transpose(1, 0, 2))
    m = {
        "xT": xT.astype(NP_BF16),
        "xr": xr.astype(NP_BF16),
        "wih": Wt.astype(NP_BF16),
        "whh": Ht.astype(NP_BF16),
        "ident": np.eye(128, dtype=np.float32),
    }
    if bih is not None:
        m["bias_i"] = bih.reshape(1, 3 * F).astype(NP_BF16)
        m["bias_h"] = bhh.reshape(1, 3 * F).astype(NP_BF16)
        m["ones"] = np.ones((1, 128), NP_BF16)
    return m


def _install_ntff_hook():
    """The agent image's antenv lacks axon_hooks; recreate it so
    run_bass_kernel_spmd(trace=True) can capture NTFF profiles."""
    import sys as _sys
    if "antenv.axon_hooks" in _sys.modules:
        return True
    so_path = "/opt/axon/libaxon_pjrt.so"
    if not os.path.exists(so_path):
        return False
    import contextlib
    import ctypes
    import types
    lib = ctypes.CDLL(so_path)
    if not hasattr(lib, "axon_start_nrt_profile"):
        return False
    lib.axon_start_nrt_profile.argtypes = [
        ctypes.POINTER(ctypes.c_int64), ctypes.c_size_t]
    lib.axon_start_nrt_profile.restype = ctypes.c_int64
    lib.axon_stop_nrt_profile.argtypes = [ctypes.c_char_p]
    lib.axon_stop_nrt_profile.restype = ctypes.c_int64

    @contextlib.contextmanager
    def _hook(output_dir, device_ids):
        import jax
        jax.devices()
        if device_ids:
            ids = (ctypes.c_int64 * len(device_ids))(*device_ids)
            rc = lib.axon_start_nrt_profile(ids, len(device_ids))
        else:
            rc = lib.axon_start_nrt_profile(None, 0)
        if rc != 0:
            raise RuntimeError(f"axon_start_nrt_profile rc={rc}")
        try:
            yield
        finally:
            n = lib.axon_stop_nrt_profile(str(output_dir).encode())
            print(f"profile: {n} file(s) written to {output_dir}",
                  file=sys.stderr)

    mod = types.ModuleType("antenv.axon_hooks")
    mod.get_axon_ntff_profile_hook = lambda: _hook
    mod.set_axon_ntff_profile_hook = lambda h: None
    _sys.modules["antenv.axon_hooks"] = mod
    return True


def _run(inputs, trace=False):
    input_x = np.asarray(inputs["input_x"], np.float32)
    Wih_f = np.asarray(inputs["Wih_f"], np.float32)
    Whh_f = np.asarray(inputs["Whh_f"], np.float32)
    Wih_b = np.asarray(inputs["Wih_b"], np.float32)
    Whh_b = np.asarray(inputs["Whh_b"], np.float32)
    bih_f = np.asarray(inputs["bih_f"], np.float32)
    bhh_f = np.asarray(inputs["bhh_f"], np.float32)
    bih_b = np.asarray(inputs["bih_b"], np.float32)
    bhh_b = np.asarray(inputs["bhh_b"], np.float32)
    L = int(inputs["L"])

    has_bias = bool(
        np.any(bih_f) or np.any(bhh_f) or np.any(bih_b) or np.any(bhh_b))
    key = (has_bias, S, CHUNK)
    if key not in _PROG_CACHE:
        _PROG_CACHE[key] = _build_program(has_bias)
    nc = _PROG_CACHE[key]

    cx = np.ascontiguousarray(input_x[:, :, :F])
    in_maps = []
    for core in range(N_CORES):
        fwd = core < N_FWD
        in_maps.append(_prep_core_inputs(
            cx,
            Wih_f if fwd else Wih_b,
            Whh_f if fwd else Whh_b,
            (bih_f if fwd else bih_b) if has_bias else None,
            (bhh_f if fwd else bhh_b) if has_bias else None,
            core,
        ))

    if trace and not _install_ntff_hook():
        trace = False
    res = run_bass_kernel_spmd(nc, in_maps, list(range(N_CORES)), trace=trace)

    # reassemble: hs[dir][b, t, F]
    hs_f = np.empty((B, T, F), np.float32)
    hs_b = np.empty((B, T, F), np.float32)
    for core in range(N_CORES):
        o = np.asarray(res.results[core]["out"]).astype(np.float32)
        o = o.reshape(CHUNK, NCH, B, F)
        o = o.transpose(1, 2, 0, 3)                    # [c, b, chunk, F]
        fwd = core < N_FWD
        k = core if fwd else core - N_FWD
        dst = hs_f if fwd else hs_b
        for c in range(NCH):
            t0 = CHUNK * (NCH * k + c)
            dst[:, t0:t0 + CHUNK, :] = o[c]
    out = np.empty((B, T - 2 * L, 2 * F), np.float32)
    out[:, :, :F] = hs_f[:, L:T - L, :]
    out[:, :, F:] = hs_b[:, L:T - L, :]
    return out, res


def kernel(**inputs) -> np.ndarray:
    out, _ = _run(inputs, trace=False)
    return out
